# revision 8
# baseline (speedup 1.0000x reference)
"""APEG block (scatter -> depthwise 3x3 conv -> gather) on 8 TRN2 NeuronCores.

Strategy (channel-sharded, 32 channels per core, zero communication):
  - host buckets tokens by row-class q = row % 4 and r4-chunk (index prep only)
  - on-device gpsimd local_scatter builds the bf16 class-layout grid
      cls[p = 32*q + ch, (r4, col)] = token value (zeros elsewhere)
  - DMA transposes class layout -> row-major per-block DRAM staging -> SBUF
    row-layout blocks Rt[p = row_in_block, ch, col] (partition permute via
    a DRAM hop, since one-hop SBUF->SBUF cannot permute partitions)
  - PE computes the depthwise conv as banded matmuls: for each (block,
    channel), 3 matmuls (one per column tap dc) with a [98 x 96] banded
    stationary encoding the 3 row taps, accumulating into PSUM
  - ACT/DVE evict PSUM (f32) to bf16 conv blocks; DMA out the full conv grid
  - host gathers conv values at the token coordinates and adds bias (f32)
"""

import os
import sys

if "/opt/trn_rl_repo" not in sys.path:
    sys.path.insert(0, "/opt/trn_rl_repo")

import numpy as np
import ml_dtypes

BF16 = ml_dtypes.bfloat16

H = W = 384
N_TOK = 65536
D = 256
Q = 4
R4 = H // 4
CELLS = R4 * W
DC = 32                 # channels per core
NCORES = D // DC
NBLK = 4
BR = H // NBLK          # 96 rows per block
KP = BR + 2
WP = W + 4              # Rt cols: 2 zero pads each side
R4C = 4                 # r4 rows per local_scatter chunk
NCHUNK = R4 // R4C      # 24
CCELLS = R4C * W        # 1536 dst cells per chunk
CGRP = 2                # chunks per input DMA group
NGRP = NCHUNK // CGRP

_last_exec_ns = None
_cache = {}


def _host_prep(tokens, coords, weight, ctok):
    rows = np.asarray(coords[:, 0], dtype=np.int64)
    cols = np.asarray(coords[:, 1], dtype=np.int64)
    q = rows % Q
    r4 = rows // Q
    chunk = r4 // R4C
    bucket = q * NCHUNK + chunk

    order = np.argsort(bucket, kind="stable")
    counts = np.bincount(bucket, minlength=Q * NCHUNK)
    assert counts.max() <= ctok
    starts = np.concatenate([[0], np.cumsum(counts)[:-1]])
    ranks = np.empty(N_TOK, dtype=np.int64)
    ranks[order] = np.arange(N_TOK) - np.repeat(starts, counts)

    data_q = np.zeros((Q, NCHUNK, ctok, D), dtype=BF16)
    data_q[q, chunk, ranks, :] = tokens.astype(BF16)
    data = np.ascontiguousarray(
        data_q.reshape(Q, NCHUNK * ctok, NCORES, DC).transpose(2, 0, 3, 1)
    ).reshape(NCORES, Q * DC, NCHUNK * ctok)

    idxb = np.full((Q, NCHUNK, ctok), -1, dtype=np.int16)
    idxb[q, chunk, ranks] = ((r4 - chunk * R4C) * W + cols).astype(np.int16)
    idxw = np.repeat(idxb.reshape(Q, 1, NCHUNK * ctok), DC, axis=1).reshape(
        Q * DC, NCHUNK * ctok)

    wb = np.asarray(weight).reshape(D, 3, 3).astype(BF16)
    stat = np.zeros((NCORES, KP, DC, 3, BR), dtype=BF16)
    m_idx = np.arange(BR)
    for dr in range(3):
        for core in range(NCORES):
            wslice = wb[core * DC:(core + 1) * DC]
            stat[core, m_idx + dr, :, :, m_idx] = wslice[None, :, dr, :]

    in_maps = []
    ngrp = NCHUNK // CGRP
    gc = CGRP * ctok
    for core in range(NCORES):
        dv = data[core].view(np.int16).reshape(Q * DC, ngrp, gc)
        iv = idxw.reshape(Q * DC, ngrp, gc)
        dix = np.ascontiguousarray(
            np.stack([dv, iv], axis=2)).reshape(Q * DC, ngrp * 2 * gc)
        in_maps.append({
            "dix": dix,
            "stat": np.ascontiguousarray(stat[core].reshape(KP, DC * 3 * BR)),
        })
    return in_maps, rows, cols


def _build_nc(ctok):
    import concourse.bacc as bacc
    import concourse.mybir as mybir
    from concourse import tile

    bf = mybir.dt.bfloat16
    HDC = DC // 2

    nc = bacc.Bacc("TRN2", target_bir_lowering=False, debug=False,
                   num_devices=NCORES)
    dix_d = nc.declare_dram_parameter("dix", [Q * DC, NCHUNK * ctok * 2],
                                      mybir.dt.int16, isOutput=False)
    stat_d = nc.declare_dram_parameter("stat", [KP, DC * 3 * BR], bf,
                                       isOutput=False)
    out_d = nc.declare_dram_parameter("out", [NBLK, BR, DC, W], bf,
                                      isOutput=True)

    with tile.TileContext(nc) as tc:
        with (
            tc.tile_pool(name="gdram", bufs=1, space="DRAM") as gpool,
            tc.tile_pool(name="consts", bufs=1) as cpool,
            tc.tile_pool(name="grid", bufs=1) as gridpool,
            tc.tile_pool(name="statp", bufs=1) as spool,
            tc.tile_pool(name="io", bufs=7) as iopool,
            tc.tile_pool(name="work", bufs=2) as wpool,
            tc.tile_pool(name="conv", bufs=2) as convpool,
            tc.tile_pool(name="psum", bufs=8, space="PSUM") as pspool,
        ):
            Gt = [gpool.tile([DC, KP, W], bf, tag=f"g{t}", name=f"g{t}")
                  for t in range(NBLK)]

            zrow_t = cpool.tile([DC, W], bf)
            # class grid split into 4 block-aligned tiles (24 r4 each) so
            # hop1(t) depends mostly on the chunks it actually reads
            R4T = R4 // NBLK
            Ct = [gridpool.tile([Q * DC, R4T * W], bf, tag=f"c{t}",
                                name=f"c{t}") for t in range(NBLK)]
            stat_t = spool.tile([KP, DC, 3, BR], bf)

            rt_bufs = [wpool.tile([128, DC, WP], bf, tag="rt", name=f"rt{i}")
                       for i in range(2)]
            for rb in rt_bufs:
                nc.vector.memset(rb[:, :, 0:2], 0.0)
                nc.vector.memset(rb[:, :, WP - 2:WP], 0.0)

            nc.vector.memset(zrow_t[:], 0.0)
            nc.sync.dma_start(Gt[0][:, 0, :], zrow_t[:])
            nc.sync.dma_start(Gt[NBLK - 1][:, KP - 1, :], zrow_t[:])
            nc.sync.dma_start(stat_t[:], stat_d.ap().rearrange(
                "k (c j m) -> k c j m", c=DC, j=3))

            def hop1(t):
                for qq in range(Q):
                    r4lo = max(0, -(-(BR * t - 1 - qq) // 4))
                    r4hi = min(R4 - 1, (BR * t + BR - qq) // 4)
                    for tt in range(r4lo // R4T, r4hi // R4T + 1):
                        s0 = max(r4lo, tt * R4T)
                        s1 = min(r4hi, tt * R4T + R4T - 1)
                        src = Ct[tt][qq * DC:(qq + 1) * DC,
                                     (s0 - tt * R4T) * W:
                                     (s1 + 1 - tt * R4T) * W].rearrange(
                            "p (r c) -> p r c", r=s1 - s0 + 1)
                        row0 = 4 * s0 + qq - BR * t + 1
                        dst = Gt[t][:, row0: row0 + 4 * (s1 - s0) + 1: 4, :]
                        nc.sync.dma_start(dst, src)

            need = [min(NCHUNK, (BR * (t + 1)) // 4 // R4C + 1)
                    for t in range(NBLK)]
            grp_tiles = {}

            gc = CGRP * ctok

            def fetch_group(g):
                gt_ = iopool.tile([Q * DC, 2 * gc], mybir.dt.int16, tag="gx",
                                  name=f"gx{g}")
                nc.scalar.dma_start(
                    gt_[:], dix_d.ap()[:, g * 2 * gc:(g + 1) * 2 * gc])
                grp_tiles[g] = (gt_[:, 0:gc].bitcast(bf), gt_[:, gc:2 * gc])

            for g in range(min(7, NGRP)):
                fetch_group(g)
            done = 0
            for t in range(NBLK):
                while done < need[t]:
                    k = done
                    g = k // CGRP
                    if g not in grp_tiles:
                        fetch_group(g)
                    dt_, it_ = grp_tiles[g]
                    kk = k % CGRP
                    tt, cc = k * R4C // R4T, (k * R4C) % R4T // R4C
                    nc.gpsimd.local_scatter(
                        Ct[tt][:, cc * CCELLS:(cc + 1) * CCELLS],
                        dt_[:, kk * ctok:(kk + 1) * ctok],
                        it_[:, kk * ctok:(kk + 1) * ctok],
                        channels=Q * DC, num_elems=CCELLS, num_idxs=ctok)
                    done += 1
                hop1(t)

                rt = rt_bufs[t % 2]
                src = Gt[t][:, :, :].transpose([1, 0, 2])
                nc.sync.dma_start(rt[0:KP, :, 2:2 + W], src)

                for half in range(4):
                    conv = convpool.tile([BR, HDC // 2, W], bf)
                    for chh in range(HDC // 2):
                        ch = half * (HDC // 2) + chh
                        ps = pspool.tile([BR, W + 2], mybir.dt.float32)
                        for dc in range(3):
                            nc.tensor.matmul(
                                ps[:],
                                stat_t[:, ch, dc, :],
                                rt[0:KP, ch, dc:dc + W + 2],
                                start=(dc == 0), stop=(dc == 2))
                        if ch % 2 == 0:
                            nc.scalar.copy(conv[:, chh, :], ps[:, 1:1 + W])
                        else:
                            nc.vector.tensor_copy(conv[:, chh, :],
                                                  ps[:, 1:1 + W])
                    nc.scalar.dma_start(
                        out_d.ap()[t, :, half * (HDC // 2):
                                   (half + 1) * (HDC // 2), :],
                        conv[:])

    nc.compile()
    return nc


def kernel(tokens, coords, weight, bias, grid_h, grid_w):
    global _last_exec_ns
    tokens = np.asarray(tokens, dtype=np.float32)
    coords = np.asarray(coords)
    weight = np.asarray(weight, dtype=np.float32)
    bias = np.asarray(bias, dtype=np.float32)
    assert int(grid_h) == H and int(grid_w) == W
    assert tokens.shape == (N_TOK, D)

    # padded bucket capacity (input-dependent; NEFF is compiled per value)
    rows = np.asarray(coords[:, 0], dtype=np.int64)
    r4 = rows // Q
    counts = np.bincount((rows % Q) * NCHUNK + r4 // R4C,
                         minlength=Q * NCHUNK)
    ctok = max(832, (int(counts.max()) + 63) // 64 * 64)

    in_maps, rows, cols = _host_prep(tokens, coords, weight, ctok)

    if ctok not in _cache:
        _cache[ctok] = _build_nc(ctok)
    nc = _cache[ctok]

    from concourse.bass_utils import run_bass_kernel_spmd
    trace = bool(os.environ.get("APEG_TRACE"))
    res = run_bass_kernel_spmd(nc, in_maps, core_ids=list(range(NCORES)),
                               trace=trace)
    _last_exec_ns = res.exec_time_ns

    outs = []
    for core in range(NCORES):
        arr = np.asarray(res.results[core]["out"]).reshape(H, DC, W)
        vals = arr[rows, :, cols].astype(np.float32)
        vals += bias[core * DC:(core + 1) * DC][None, :]
        outs.append(vals)
    # reference returns [D, N]
    return np.ascontiguousarray(np.concatenate(outs, axis=1).T)


# revision 9
# speedup vs baseline: 1.1554x; 1.1554x over previous
"""APEG block (scatter -> depthwise 3x3 conv -> gather) on 8 TRN2 NeuronCores.

Strategy (channel-sharded, 32 channels per core, zero communication):
  - host buckets tokens by row-class q = row % 4 and r4-chunk (index prep only)
  - on-device gpsimd local_scatter builds the bf16 class-layout grid
      cls[p = 32*q + ch, (r4, col)] = token value (zeros elsewhere)
  - DMA transposes class layout -> row-major per-block DRAM staging -> SBUF
    row-layout blocks Rt[p = row_in_block, ch, col] (partition permute via
    a DRAM hop, since one-hop SBUF->SBUF cannot permute partitions)
  - PE computes the depthwise conv as banded matmuls: for each (block,
    channel), 3 matmuls (one per column tap dc) with a [98 x 96] banded
    stationary encoding the 3 row taps, accumulating into PSUM
  - ACT/DVE evict PSUM (f32) to bf16 conv blocks; DMA out the full conv grid
  - host gathers conv values at the token coordinates and adds bias (f32)
"""

import os
import sys

if "/opt/trn_rl_repo" not in sys.path:
    sys.path.insert(0, "/opt/trn_rl_repo")

import numpy as np
import ml_dtypes

BF16 = ml_dtypes.bfloat16

H = W = 384
N_TOK = 65536
D = 256
Q = 4
R4 = H // 4
CELLS = R4 * W
DC = 32                 # channels per core
NCORES = D // DC
NBLK = 4
BR = H // NBLK          # 96 rows per block
KP = BR + 2
WP = W + 4              # Rt cols: 2 zero pads each side
R4C = 4                 # r4 rows per local_scatter chunk
NCHUNK = R4 // R4C      # 24
CCELLS = R4C * W        # 1536 dst cells per chunk
CGRP = 2                # chunks per input DMA group
NGRP = NCHUNK // CGRP

_last_exec_ns = None
_cache = {}


def _host_prep(tokens, coords, weight, ctok):
    rows = np.asarray(coords[:, 0], dtype=np.int64)
    cols = np.asarray(coords[:, 1], dtype=np.int64)
    q = rows % Q
    r4 = rows // Q
    chunk = r4 // R4C
    bucket = q * NCHUNK + chunk

    order = np.argsort(bucket, kind="stable")
    counts = np.bincount(bucket, minlength=Q * NCHUNK)
    assert counts.max() <= ctok
    starts = np.concatenate([[0], np.cumsum(counts)[:-1]])
    ranks = np.empty(N_TOK, dtype=np.int64)
    ranks[order] = np.arange(N_TOK) - np.repeat(starts, counts)

    data_q = np.zeros((Q, NCHUNK, ctok, D), dtype=BF16)
    data_q[q, chunk, ranks, :] = tokens.astype(BF16)
    data = np.ascontiguousarray(
        data_q.reshape(Q, NCHUNK * ctok, NCORES, DC).transpose(2, 0, 3, 1)
    ).reshape(NCORES, Q * DC, NCHUNK * ctok)

    idxb = np.full((Q, NCHUNK, ctok), -1, dtype=np.int16)
    idxb[q, chunk, ranks] = ((r4 - chunk * R4C) * W + cols).astype(np.int16)
    idxw = np.repeat(idxb.reshape(Q, 1, NCHUNK * ctok), DC, axis=1).reshape(
        Q * DC, NCHUNK * ctok)

    wb = np.asarray(weight).reshape(D, 3, 3).astype(BF16)
    stat = np.zeros((NCORES, KP, DC, 3, BR), dtype=BF16)
    m_idx = np.arange(BR)
    for dr in range(3):
        for core in range(NCORES):
            wslice = wb[core * DC:(core + 1) * DC]
            stat[core, m_idx + dr, :, :, m_idx] = wslice[None, :, dr, :]

    in_maps = []
    ngrp = NCHUNK // CGRP
    gc = CGRP * ctok
    for core in range(NCORES):
        dv = data[core].view(np.int16).reshape(Q * DC, ngrp, gc)
        iv = idxw.reshape(Q * DC, ngrp, gc)
        dix = np.ascontiguousarray(
            np.stack([dv, iv], axis=2)).reshape(Q * DC, ngrp * 2 * gc)
        in_maps.append({
            "dix": dix,
            "stat": np.ascontiguousarray(stat[core].reshape(KP, DC * 3 * BR)),
        })
    return in_maps, rows, cols


def _build_nc(ctok):
    import concourse.bacc as bacc
    import concourse.mybir as mybir
    from concourse import tile

    bf = mybir.dt.bfloat16
    HDC = DC // 2

    nc = bacc.Bacc("TRN2", target_bir_lowering=False, debug=False,
                   num_devices=NCORES)
    dix_d = nc.declare_dram_parameter("dix", [Q * DC, NCHUNK * ctok * 2],
                                      mybir.dt.int16, isOutput=False)
    stat_d = nc.declare_dram_parameter("stat", [KP, DC * 3 * BR], bf,
                                       isOutput=False)
    out_d = nc.declare_dram_parameter("out", [NBLK, BR, DC, W], bf,
                                      isOutput=True)

    with tile.TileContext(nc) as tc:
        with (
            tc.tile_pool(name="gdram", bufs=1, space="DRAM") as gpool,
            tc.tile_pool(name="consts", bufs=1) as cpool,
            tc.tile_pool(name="grid", bufs=1) as gridpool,
            tc.tile_pool(name="statp", bufs=1) as spool,
            tc.tile_pool(name="io", bufs=7) as iopool,
            tc.tile_pool(name="work", bufs=2) as wpool,
            tc.tile_pool(name="conv", bufs=2) as convpool,
            tc.tile_pool(name="psum", bufs=8, space="PSUM") as pspool,
        ):
            Gt = [gpool.tile([DC, KP, W], bf, tag=f"g{t}", name=f"g{t}")
                  for t in range(NBLK)]

            zrow_t = cpool.tile([DC, W], bf)
            # class-grid tiles with boundaries chosen so hop1(t)'s read
            # range [24t-1, 24t+24] ends exactly at a tile boundary: the
            # (tile-coarse) RAW dep then matches exactly the chunks needed
            CTB = [0, 28, 52, 76, 96]
            Ct = [gridpool.tile([Q * DC, (CTB[i + 1] - CTB[i]) * W], bf,
                                tag=f"c{i}", name=f"c{i}")
                  for i in range(len(CTB) - 1)]

            def ct_tile_of(r4v):
                for i in range(len(CTB) - 1):
                    if r4v < CTB[i + 1]:
                        return i
                raise AssertionError
            stat_t = spool.tile([KP, DC, 3, BR], bf)

            rt_bufs = [wpool.tile([128, DC, WP], bf, tag="rt", name=f"rt{i}")
                       for i in range(2)]
            for rb in rt_bufs:
                nc.vector.memset(rb[:, :, 0:2], 0.0)
                nc.vector.memset(rb[:, :, WP - 2:WP], 0.0)

            nc.vector.memset(zrow_t[:], 0.0)
            nc.sync.dma_start(Gt[0][:, 0, :], zrow_t[:])
            nc.sync.dma_start(Gt[NBLK - 1][:, KP - 1, :], zrow_t[:])
            nc.sync.dma_start(stat_t[:], stat_d.ap().rearrange(
                "k (c j m) -> k c j m", c=DC, j=3))

            def hop1(t):
                for qq in range(Q):
                    r4lo = max(0, -(-(BR * t - 1 - qq) // 4))
                    r4hi = min(R4 - 1, (BR * t + BR - qq) // 4)
                    for tt in range(ct_tile_of(r4lo), ct_tile_of(r4hi) + 1):
                        s0 = max(r4lo, CTB[tt])
                        s1 = min(r4hi, CTB[tt + 1] - 1)
                        src = Ct[tt][qq * DC:(qq + 1) * DC,
                                     (s0 - CTB[tt]) * W:
                                     (s1 + 1 - CTB[tt]) * W].rearrange(
                            "p (r c) -> p r c", r=s1 - s0 + 1)
                        row0 = 4 * s0 + qq - BR * t + 1
                        dst = Gt[t][:, row0: row0 + 4 * (s1 - s0) + 1: 4, :]
                        nc.sync.dma_start(dst, src)

            need = [min(NCHUNK, (BR * (t + 1)) // 4 // R4C + 1)
                    for t in range(NBLK)]
            grp_tiles = {}

            gc = CGRP * ctok

            def fetch_group(g):
                gt_ = iopool.tile([Q * DC, 2 * gc], mybir.dt.int16, tag="gx",
                                  name=f"gx{g}")
                nc.scalar.dma_start(
                    gt_[:], dix_d.ap()[:, g * 2 * gc:(g + 1) * 2 * gc])
                grp_tiles[g] = (gt_[:, 0:gc].bitcast(bf), gt_[:, gc:2 * gc])

            for g in range(min(7, NGRP)):
                fetch_group(g)
            done = 0
            for t in range(NBLK):
                while done < need[t]:
                    k = done
                    g = k // CGRP
                    if g not in grp_tiles:
                        fetch_group(g)
                    dt_, it_ = grp_tiles[g]
                    kk = k % CGRP
                    tt = ct_tile_of(k * R4C)
                    off = (k * R4C - CTB[tt]) * W
                    nc.gpsimd.local_scatter(
                        Ct[tt][:, off:off + CCELLS],
                        dt_[:, kk * ctok:(kk + 1) * ctok],
                        it_[:, kk * ctok:(kk + 1) * ctok],
                        channels=Q * DC, num_elems=CCELLS, num_idxs=ctok)
                    done += 1
                hop1(t)

                rt = rt_bufs[t % 2]
                src = Gt[t][:, :, :].transpose([1, 0, 2])
                nc.sync.dma_start(rt[0:KP, :, 2:2 + W], src)

                for half in range(4):
                    conv = convpool.tile([BR, HDC // 2, W], bf)
                    for chh in range(HDC // 2):
                        ch = half * (HDC // 2) + chh
                        ps = pspool.tile([BR, W + 2], mybir.dt.float32)
                        for dc in range(3):
                            nc.tensor.matmul(
                                ps[:],
                                stat_t[:, ch, dc, :],
                                rt[0:KP, ch, dc:dc + W + 2],
                                start=(dc == 0), stop=(dc == 2))
                        if ch % 2 == 0:
                            nc.scalar.copy(conv[:, chh, :], ps[:, 1:1 + W])
                        else:
                            nc.vector.tensor_copy(conv[:, chh, :],
                                                  ps[:, 1:1 + W])
                    nc.scalar.dma_start(
                        out_d.ap()[t, :, half * (HDC // 2):
                                   (half + 1) * (HDC // 2), :],
                        conv[:])

    nc.compile()
    return nc


def kernel(tokens, coords, weight, bias, grid_h, grid_w):
    global _last_exec_ns
    tokens = np.asarray(tokens, dtype=np.float32)
    coords = np.asarray(coords)
    weight = np.asarray(weight, dtype=np.float32)
    bias = np.asarray(bias, dtype=np.float32)
    assert int(grid_h) == H and int(grid_w) == W
    assert tokens.shape == (N_TOK, D)

    # padded bucket capacity (input-dependent; NEFF is compiled per value)
    rows = np.asarray(coords[:, 0], dtype=np.int64)
    r4 = rows // Q
    counts = np.bincount((rows % Q) * NCHUNK + r4 // R4C,
                         minlength=Q * NCHUNK)
    ctok = max(832, (int(counts.max()) + 63) // 64 * 64)

    in_maps, rows, cols = _host_prep(tokens, coords, weight, ctok)

    if ctok not in _cache:
        _cache[ctok] = _build_nc(ctok)
    nc = _cache[ctok]

    from concourse.bass_utils import run_bass_kernel_spmd
    trace = bool(os.environ.get("APEG_TRACE"))
    res = run_bass_kernel_spmd(nc, in_maps, core_ids=list(range(NCORES)),
                               trace=trace)
    _last_exec_ns = res.exec_time_ns

    outs = []
    for core in range(NCORES):
        arr = np.asarray(res.results[core]["out"]).reshape(H, DC, W)
        vals = arr[rows, :, cols].astype(np.float32)
        vals += bias[core * DC:(core + 1) * DC][None, :]
        outs.append(vals)
    # reference returns [D, N]
    return np.ascontiguousarray(np.concatenate(outs, axis=1).T)
